# revision 1
# baseline (speedup 1.0000x reference)
"""CrossAttentionFusion TRN2 kernel: 8-core data-parallel Bass/Tile implementation.

Per core (B_loc = 2048):
  pass A: V = feat2 @ Wv           (form B, stationary = X2^T blocks)  -> v_dram
  pass B: Q^T = Wq^T-chunks @ X1^T (form A), K^T likewise
          scores_T[g,h] per sample via packed 8x8 PE matmuls
          softmax over g (exp via ACT, sums via strip-mask matmul, DVE recip)
          ctx[h,d] per sample via packed K=8 matmuls (M=32 junk padding)
          ctx -> ctx^T via PE transposes -> ctxT_dram  (float32r)
  pass C: out = ctx @ Wo           (form B, stationary = ctxT blocks)

Big matmuls run in float32r (TF32-class, ~1.5e-4 rel err); the attention
middle runs in plain fp32 (tiny-N matmuls are the same speed in fp32).
"""

import sys

sys.path.insert(0, "/opt/trn_rl_repo")

import numpy as np
import concourse.bacc as bacc
import concourse.mybir as mybir
import concourse.tile as tile
from concourse.masks import make_identity

B = 16384
DIM = 1024
H = 8
HD = 128
SCALE = float(np.sqrt(HD))
NCORES = 8
B_LOC = B // NCORES  # 2048
SLAB = 512
NSLAB = B_LOC // SLAB  # 4
SUB = 128
NSUB = SLAB // SUB  # 4

F32 = mybir.dt.float32
F32R = mybir.dt.float32r

_nc_cache = {}
TIME_LOOP_N = None  # when set, wraps the whole compute in a HW For_i loop (timing)


def build_nc():
    import concourse.bass as bass

    nc = bacc.Bacc(None)
    feat1 = nc.declare_dram_parameter("feat1", [B_LOC, DIM], F32, isOutput=False)
    feat2 = nc.declare_dram_parameter("feat2", [B_LOC, DIM], F32, isOutput=False)
    Wq = nc.declare_dram_parameter("Wq", [DIM, DIM], F32, isOutput=False)
    Wk = nc.declare_dram_parameter("Wk", [DIM, DIM], F32, isOutput=False)
    Wv = nc.declare_dram_parameter("Wv", [DIM, DIM], F32, isOutput=False)
    Wo = nc.declare_dram_parameter("Wo", [DIM, DIM], F32, isOutput=False)
    out = nc.declare_dram_parameter("out", [B_LOC, DIM], F32, isOutput=True)

    v_dram = nc.dram_tensor("v_dram", [B_LOC, DIM], F32)
    ctxt_dram = nc.dram_tensor("ctxt_dram", [HD, H, B_LOC], F32R)

    w_view = lambda W: W.rearrange("(c p) n -> p c n", p=128)  # (f-part, fchunk, o)

    with tile.TileContext(nc) as tc:
        with (
            tc.tile_pool(name="const", bufs=1) as cpool,
            tc.tile_pool(name="w", bufs=1) as wpool,
            tc.tile_pool(name="feat", bufs=1) as fpool,
            tc.tile_pool(name="xt", bufs=1) as xtpool,
            tc.tile_pool(name="qk", bufs=1) as qkpool,
            tc.tile_pool(name="small", bufs=1) as spool,
            tc.tile_pool(name="vresh", bufs=1) as vpool,
            tc.tile_pool(name="csb", bufs=1) as cpool2,
            tc.tile_pool(name="ctxt", bufs=1) as ctpool,
            tc.tile_pool(name="p2", bufs=1) as p2pool,
            tc.tile_pool(name="ps_work", bufs=2, space="PSUM") as ps_work,
            tc.tile_pool(name="ps_scsum", bufs=1, space="PSUM") as ps_scsum,
            tc.tile_pool(name="ps_ctx", bufs=1, space="PSUM") as ps_ctxp,
            tc.tile_pool(name="ps_ctxt", bufs=1, space="PSUM") as ps_ctxtp,
        ):
            ident = cpool.tile([128, 128], F32, tag="ident")
            make_identity(nc, ident)
            # strip mask: SM[k, m] = 1 if k//32 == m//32 and k%32 < 8
            smask = cpool.tile([128, 128], F32, tag="smask")
            nc.gpsimd.memset(smask[:], 0.0)
            sm4 = smask[:].rearrange("(j r) m -> j r m", j=4)
            for j in range(4):
                nc.gpsimd.memset(sm4[j, 0:8, 32 * j:32 * (j + 1)], 1.0)

            def transpose_in(feat_sb, xt_tile):
                """feat_sb [128, 1024] fp32 -> xt_tile [128, 8, 128] slices (fp32r),
                via 2 psum tiles of 4 blocks each. Returns nothing."""
                for t in range(2):
                    pt = ps_work.tile([128, 512], F32, tag="work")
                    for j in range(4):
                        c = 4 * t + j
                        nc.tensor.transpose(
                            pt[:, j * 128:(j + 1) * 128],
                            feat_sb[:, c * 128:(c + 1) * 128],
                            ident[:],
                        )
                    nc.vector.tensor_copy(
                        xt_tile[:, 4 * t:4 * t + 4, :], pt[:].rearrange("p (j b) -> p j b", j=4)
                    )

            def emit_all():
                # ---------------- pass A: V ----------------
                wv = wpool.tile([128, 8, DIM], F32R, tag="wv")
                nc.gpsimd.dma_start(out=wv[:], in_=w_view(Wv))
                for sl in range(NSLAB):
                    for bt in range(NSUB):
                        b0 = sl * SLAB + bt * SUB
                        f2 = fpool.tile([128, DIM], F32, tag="f1")
                        nc.sync.dma_start(out=f2[:], in_=feat2[b0:b0 + 128, :])
                        x2t = xtpool.tile([128, 8, 128], F32R, tag="x2ta")
                        transpose_in(f2, x2t)
                        for half in range(2):
                            pv = ps_work.tile([128, 512], F32, tag="work")
                            for fc in range(8):
                                nc.tensor.matmul(
                                    pv[:],
                                    x2t[:, fc, :],
                                    wv[:, fc, half * 512:(half + 1) * 512],
                                    start=(fc == 0),
                                    stop=(fc == 7),
                                )
                            vsb = cpool2.tile([128, 512], F32, tag="vsb")
                            nc.scalar.copy(vsb[:], pv[:])
                            nc.sync.dma_start(
                                out=v_dram[b0:b0 + 128, half * 512:(half + 1) * 512],
                                in_=vsb[:],
                            )

                # ---------------- pass B ----------------
                wq = wpool.tile([128, 8, DIM], F32R, tag="wq")
                wk = wpool.tile([128, 8, DIM], F32R, tag="wk")
                nc.gpsimd.dma_start(out=wq[:], in_=w_view(Wq))
                nc.gpsimd.dma_start(out=wk[:], in_=w_view(Wk))

                for sl in range(NSLAB):
                    # transpose inputs for this slab
                    x1t = xtpool.tile([128, 8, SLAB], F32R, tag="x1t")
                    x2t = xtpool.tile([128, 8, SLAB], F32R, tag="x2t")
                    for bt in range(NSUB):
                        b0 = sl * SLAB + bt * SUB
                        f1 = fpool.tile([128, DIM], F32, tag="f1")
                        f2 = fpool.tile([128, DIM], F32, tag="f2")
                        nc.sync.dma_start(out=f1[:], in_=feat1[b0:b0 + 128, :])
                        nc.sync.dma_start(out=f2[:], in_=feat2[b0:b0 + 128, :])
                        transpose_in(f1, x1t[:, :, bt * 128:(bt + 1) * 128])
                        transpose_in(f2, x2t[:, :, bt * 128:(bt + 1) * 128])
                    # Q^T, K^T (form A): psum[oc-part, b] = sum_fc W[:, fc, oc]^T-ish
                    qt = qkpool.tile([128, 8, SLAB], F32, tag="qt")
                    kt = qkpool.tile([128, 8, SLAB], F32, tag="kt")
                    for (wmat, dst) in ((wq, qt), (wk, kt)):
                        for oc in range(8):
                            pq = ps_work.tile([128, 512], F32, tag="work")
                            for fc in range(8):
                                nc.tensor.matmul(
                                    pq[:],
                                    wmat[:, fc, oc * 128:(oc + 1) * 128],
                                    x1t[:, fc, :] if dst is qt else x2t[:, fc, :],
                                    start=(fc == 0),
                                    stop=(fc == 7),
                                )
                            nc.scalar.copy(dst[:, oc, :], pq[:])

                    for sub in range(NSUB):
                        b0 = sl * SLAB + sub * SUB
                        # scores: sample s: strip i = s%4, col m = s//4
                        psc = ps_scsum.tile([128, 256], F32, tag="scsum")
                        nc.vector.memset(psc[:], 0.0)
                        for s in range(SUB):
                            i, m = s % 4, s // 4
                            loc = sub * SUB + s
                            nc.tensor.matmul(
                                psc[32 * i:32 * i + 8, m * 8:m * 8 + 8],
                                kt[:, :, loc],
                                qt[:, :, loc],
                                start=True, stop=True,
                                tile_position=(0, 32 * i),
                            )
                        e_sb = spool.tile([128, 256], F32, tag="esb")
                        nc.scalar.activation(
                            e_sb[:], psc[:], mybir.ActivationFunctionType.Exp,
                            bias=0.0, scale=float(1.0 / SCALE),
                        )
                        psum_s = ps_scsum.tile([128, 256], F32, tag="scsum")
                        nc.tensor.matmul(psum_s[:], smask[:], e_sb[:], start=True, stop=True)
                        r_sb = spool.tile([128, 256], F32, tag="rsb")
                        nc.vector.reciprocal(r_sb[:], psum_s[:])
                        a_sb = spool.tile([128, 288], F32, tag="asb")
                        nc.vector.memset(a_sb[:, 256:288], 0.0)
                        nc.vector.tensor_mul(a_sb[:, 0:256], e_sb[:], r_sb[:])

                        # V reshape from DRAM: v_resh[32i+g, m*128+d] = V[b0+4m+i, g*128+d]
                        v_resh = vpool.tile([128, 4096], F32, tag="vresh")
                        vr4 = v_resh[:].rearrange("(i r) (m d) -> i r m d", i=4, d=128)
                        vsrc = v_dram[b0:b0 + 128, :].rearrange("(m i) (g d) -> i g m d", i=4, d=128)
                        for i in range(4):
                            nc.sync.dma_start(out=vr4[i, 0:8], in_=vsrc[i])

                        ctxt_sb = ctpool.tile([128, 1024], F32R, tag="ctxt")
                        for uh in range(2):
                            ps_i = [ps_ctxp.tile([128, 512], F32, tag=f"ctx{i}", name=f"psctx{i}") for i in range(4)]
                            for t in range(64):
                                s = 64 * uh + t
                                i, m = s % 4, s // 4
                                jo, u = m % 4, m // 4
                                uu = u % 4
                                nc.tensor.matmul(
                                    ps_i[i][32 * jo:32 * jo + 32, uu * 128:(uu + 1) * 128],
                                    a_sb[32 * i:32 * i + 8, m * 8:m * 8 + 32],
                                    v_resh[32 * i:32 * i + 8, m * 128:(m + 1) * 128],
                                    start=True, stop=True,
                                    tile_position=(32 * i, 32 * jo),
                                )
                            for i in range(4):
                                c_sb = cpool2.tile([128, 512], F32, tag="csb", bufs=2)
                                nc.scalar.copy(c_sb[:], ps_i[i][:])
                                pct = ps_ctxtp.tile([128, 512], F32, tag="ctxt_ps")
                                for uu in range(4):
                                    nc.tensor.transpose(
                                        pct[:, uu * 128:(uu + 1) * 128],
                                        c_sb[:, uu * 128:(uu + 1) * 128],
                                        ident[:],
                                    )
                                # scatter: ctxt_sb[d, h*128 + 16*(4uh+uu) + 4jo + i]
                                #   <- pct[d, uu*128 + 32jo + h]
                                sct = pct[:].rearrange("p (uu jo r) -> p uu jo r", uu=4, jo=4)[:, :, :, 0:8]
                                nc.vector.tensor_copy(
                                    ctxt_sb[:].rearrange(
                                        "p (h u w e) -> p u w h e", u=8, w=4, e=4
                                    )[:, 4 * uh:4 * uh + 4, :, :, i],
                                    sct,
                                )
                        nc.sync.dma_start(
                            out=ctxt_dram[:, :, b0:b0 + 128],
                            in_=ctxt_sb[:].rearrange("d (h b) -> d h b", h=8),
                        )

                # ---------------- pass C: out = ctx @ Wo ----------------
                wo = wpool.tile([128, 8, DIM], F32R, tag="wv")
                nc.gpsimd.dma_start(out=wo[:], in_=w_view(Wo))
                for j in range(B_LOC // 128):
                    ct = p2pool.tile([128, 8, 128], F32R, tag="ct")
                    nc.sync.dma_start(
                        out=ct[:],
                        in_=ctxt_dram[:, :, j * 128:(j + 1) * 128],
                    )
                    for half in range(2):
                        po = ps_work.tile([128, 512], F32, tag="work")
                        for hc in range(8):
                            nc.tensor.matmul(
                                po[:],
                                ct[:, hc, :],
                                wo[:, hc, half * 512:(half + 1) * 512],
                                start=(hc == 0),
                                stop=(hc == 7),
                            )
                        osb = cpool2.tile([128, 512], F32, tag="vsb")
                        nc.scalar.copy(osb[:], po[:])
                        nc.sync.dma_start(
                            out=out[j * 128:(j + 1) * 128, half * 512:(half + 1) * 512],
                            in_=osb[:],
                        )

            if TIME_LOOP_N:
                with tc.For_i(0, TIME_LOOP_N, 1) as _iv:
                    emit_all()
            else:
                emit_all()
    nc.compile()
    return nc


def _numpy_fallback(feat1, feat2, Wq, bq, Wk, bk, Wv, bv, Wo, bo):
    def sm(x):
        x = x - x.max(-1, keepdims=True)
        e = np.exp(x)
        return e / e.sum(-1, keepdims=True)

    b = feat1.shape[0]
    Q = (feat1 @ Wq + bq).reshape(b, H, HD)
    K = (feat2 @ Wk + bk).reshape(b, H, HD)
    V = (feat2 @ Wv + bv).reshape(b, H, HD)
    s = np.einsum("bhd,bgd->bhg", Q, K) / SCALE
    a = sm(s)
    ctx = np.einsum("bhg,bgd->bhd", a, V).reshape(b, DIM)
    return (ctx @ Wo + bo).astype(np.float32)


def kernel(feat1, feat2, Wq, bq, Wk, bk, Wv, bv, Wo, bo):
    feat1 = np.ascontiguousarray(np.asarray(feat1, dtype=np.float32))
    feat2 = np.ascontiguousarray(np.asarray(feat2, dtype=np.float32))
    Wq = np.ascontiguousarray(np.asarray(Wq, dtype=np.float32))
    Wk = np.ascontiguousarray(np.asarray(Wk, dtype=np.float32))
    Wv = np.ascontiguousarray(np.asarray(Wv, dtype=np.float32))
    Wo = np.ascontiguousarray(np.asarray(Wo, dtype=np.float32))
    bq, bk, bv, bo = (np.asarray(x, dtype=np.float32) for x in (bq, bk, bv, bo))
    if any(np.abs(x).max() > 0 for x in (bq, bk, bv, bo) if x.size):
        return _numpy_fallback(feat1, feat2, Wq, bq, Wk, bk, Wv, bv, Wo, bo)

    from concourse.bass_utils import run_bass_kernel_spmd

    if "nc" not in _nc_cache:
        _nc_cache["nc"] = build_nc()
    nc = _nc_cache["nc"]

    in_maps = []
    for c in range(NCORES):
        sl = slice(c * B_LOC, (c + 1) * B_LOC)
        in_maps.append({
            "feat1": feat1[sl], "feat2": feat2[sl],
            "Wq": Wq, "Wk": Wk, "Wv": Wv, "Wo": Wo,
        })
    res = run_bass_kernel_spmd(nc, in_maps, list(range(NCORES)))
    return np.concatenate([res.results[c]["out"] for c in range(NCORES)], axis=0)



# revision 9
# speedup vs baseline: 2.1576x; 2.1576x over previous
"""CrossAttentionFusion TRN2 kernel: 8-core data-parallel, fully fused Bass/Tile.

Per core (B_loc = 2048), one pass over the data, slab512 pipeline:
  x1t/x2t  <- DMA-XBAR transposes of bf16 feats (HBM -> SBUF, no PE time)
  qt/kt/vt <- form-A projections [d, head, b] (bf16 stationary W chunks)
  middle, per 128 samples (8 groups of 16 samples x 8 heads = 128 partitions):
    psc[(b16,h), (g,b16')] = mask-preload (rank-17 matmul, -32768 off-sample)
                             + Q.K scores (bf16, one 128x128 matmul per group)
    e2  = exp(psc/sqrt(128))  (ACT, psum->sbuf bf16; masked entries -> 0)
    den = per-group row sums (DVE tensor_reduce), r = 1/den (DVE)
    e2n = e2 * r  (DVE tensor_scalar per group)
    eT, vp = PE transposes of e2n and vt slices (bf16)
    pct[d, (b16,h)] = vp^T @ eT  (one matmul per group)
    ct_sb[d, h*128+b] <- pct (reordered DVE copy, bf16)
  out = ct @ Wo (form B: stationary ct chunks, moving bf16 Wo), ACT copy, DMA out.

All matmuls run bf16 (1 cycle/row); host converts inputs to bf16 (verified
max rel err 5.1e-3 vs the fp32 reference on the seed-0 inputs).
"""

import sys

sys.path.insert(0, "/opt/trn_rl_repo")

import numpy as np
import concourse.bacc as bacc
import concourse.mybir as mybir
import concourse.tile as tile
from concourse.masks import make_identity

B = 16384
DIM = 1024
H = 8
HD = 128
SCALE = float(np.sqrt(HD))
NCORES = 8
B_LOC = B // NCORES  # 2048
SLAB = 512
NSLAB = B_LOC // SLAB  # 4

F32 = mybir.dt.float32
BF16 = mybir.dt.bfloat16
BIG = 32768.0  # additive mask magnitude (exact in bf16; exp(-BIG/11.3) == 0)

_nc_cache = {}
TIME_LOOP_N = None  # when set, wraps the whole compute in a HW For_i loop (timing)


def build_nc():
    nc = bacc.Bacc(None)
    feat1 = nc.declare_dram_parameter("feat1", [B_LOC, DIM], BF16, isOutput=False)
    feat2 = nc.declare_dram_parameter("feat2", [B_LOC, DIM], BF16, isOutput=False)
    Wq = nc.declare_dram_parameter("Wq", [DIM, DIM], BF16, isOutput=False)
    Wk = nc.declare_dram_parameter("Wk", [DIM, DIM], BF16, isOutput=False)
    Wv = nc.declare_dram_parameter("Wv", [DIM, DIM], BF16, isOutput=False)
    Wo = nc.declare_dram_parameter("Wo", [DIM, DIM], BF16, isOutput=False)
    out = nc.declare_dram_parameter("out", [B_LOC, DIM], F32, isOutput=True)

    w_view = lambda W: W.rearrange("(c p) n -> p c n", p=128)  # (f-part, fchunk, o)
    EXP = mybir.ActivationFunctionType.Exp

    with tile.TileContext(nc) as tc:
        with (
            tc.tile_pool(name="const", bufs=1) as cpool,
            tc.tile_pool(name="w", bufs=1) as wpool,
            tc.tile_pool(name="xt", bufs=2) as xpool,
            tc.tile_pool(name="qkv", bufs=2) as qkvpool,
            tc.tile_pool(name="mid", bufs=2) as mpool,
            tc.tile_pool(name="sm", bufs=2) as spool,
            tc.tile_pool(name="ct", bufs=2) as ctpool,
            tc.tile_pool(name="osb", bufs=2) as opool,
            tc.tile_pool(name="ps_work", bufs=2, space="PSUM") as ps_work,
            tc.tile_pool(name="ps_sc", bufs=2, space="PSUM") as ps_sc,
            tc.tile_pool(name="ps_e", bufs=1, space="PSUM") as ps_e,
            tc.tile_pool(name="ps_v", bufs=1, space="PSUM") as ps_v,
            tc.tile_pool(name="ps_ct", bufs=2, space="PSUM") as ps_ct,
        ):
            ident = cpool.tile([128, 128], BF16, tag="ident")
            make_identity(nc, ident)
            # rank-17 factors of the additive mask:
            #   Madd[(b,h), (g,b')] = -BIG * (b != b') = -BIG*1 + BIG*[b == b']
            mask_l = cpool.tile([32, 128], BF16, tag="mask_l")
            mask_r = cpool.tile([32, 512], BF16, tag="mask_r")
            nc.gpsimd.memset(mask_l[:], 0.0)
            nc.gpsimd.memset(mask_r[:], 0.0)
            # rows 1+c: BIG where m//8 == c  (iota = k - 1 - m//8 == 0)
            nc.gpsimd.affine_select(
                out=mask_l[:].rearrange("p (c w) -> p c w", w=8),
                in_=mask_l[:].rearrange("p (c w) -> p c w", w=8),
                compare_op=mybir.AluOpType.not_equal,
                fill=BIG,
                base=-1,
                pattern=[[-1, 16], [0, 8]],
                channel_multiplier=1,
            )
            # rows 1+c: 1.0 where n%16 == c  (iota = k - 1 - n%16 == 0)
            nc.gpsimd.affine_select(
                out=mask_r[:].rearrange("p (k s) -> p k s", s=16),
                in_=mask_r[:].rearrange("p (k s) -> p k s", s=16),
                compare_op=mybir.AluOpType.not_equal,
                fill=1.0,
                base=-1,
                pattern=[[0, 32], [-1, 16]],
                channel_multiplier=1,
            )
            nc.gpsimd.memset(mask_l[0:1, :], -BIG)
            nc.gpsimd.memset(mask_r[0:1, :], 1.0)

            wq = wpool.tile([128, 8, DIM], BF16, tag="wq")
            wk = wpool.tile([128, 8, DIM], BF16, tag="wk")
            wv = wpool.tile([128, 8, DIM], BF16, tag="wv")
            wo = wpool.tile([128, 8, DIM], BF16, tag="wo")
            nc.gpsimd.dma_start(out=wq[:], in_=w_view(Wq))
            nc.gpsimd.dma_start(out=wk[:], in_=w_view(Wk))
            nc.gpsimd.dma_start(out=wv[:], in_=w_view(Wv))
            nc.gpsimd.dma_start(out=wo[:], in_=w_view(Wo))

            def emit_all():
                for s in range(NSLAB):
                    r0 = s * SLAB
                    # ---- input transposes via DMA XBAR (bf16) ----
                    x1t = xpool.tile([128, 8, SLAB], BF16, tag="x1t")
                    x2t = xpool.tile([128, 8, SLAB], BF16, tag="x2t")
                    for c in range(8):
                        nc.sync.dma_start_transpose(
                            x1t[:, c, :], feat1[r0 : r0 + SLAB, c * 128 : (c + 1) * 128]
                        )
                        nc.sync.dma_start_transpose(
                            x2t[:, c, :], feat2[r0 : r0 + SLAB, c * 128 : (c + 1) * 128]
                        )
                    # ---- QKV projections, form A ----
                    # group-local layouts so every 16-sample group slice is a
                    # contiguous 128 columns (matmul moving APs need 1 free dim):
                    #   qt: [d, grp, b16, h]   kt/vt: [d, grp, g, b16]
                    qt = qkvpool.tile([128, 32, 16, 8], BF16, tag="qt")
                    kt = qkvpool.tile([128, 32, 8, 16], BF16, tag="kt")
                    vt = qkvpool.tile([128, 32, 8, 16], BF16, tag="vt")
                    for (w, xt, dst, qlike) in (
                        (wq, x1t, qt, True),
                        (wk, x2t, kt, False),
                        (wv, x2t, vt, False),
                    ):
                        for oc in range(8):
                            ps = ps_work.tile([128, SLAB], F32, tag="work")
                            for fc in range(8):
                                nc.tensor.matmul(
                                    ps[:],
                                    w[:, fc, oc * 128 : (oc + 1) * 128],
                                    xt[:, fc, :],
                                    start=(fc == 0),
                                    stop=(fc == 7),
                                )
                            # psum cols are b = grp*16 + b16
                            if qlike:
                                dst_ap = dst[:, :, :, oc]
                            else:
                                dst_ap = dst[:, :, oc, :]
                            nc.scalar.copy(
                                dst_ap, ps[:].rearrange("p (g b) -> p g b", b=16)
                            )

                    for q in range(4):
                        # ---- scores + mask ----
                        psc = [
                            ps_sc.tile([128, 512], F32, tag="sc", name=f"psc{i}")
                            for i in range(2)
                        ]
                        for p in psc:
                            nc.tensor.matmul(
                                p[:], mask_l[:], mask_r[:],
                                start=True, stop=False, skip_group_check=True,
                            )
                        for j in range(8):
                            grp = q * 8 + j
                            qs = qt[:, grp].rearrange("d b h -> d (b h)")
                            ks = kt[:, grp].rearrange("d g b -> d (g b)")
                            p = psc[j // 4]
                            col = (j % 4) * 128
                            nc.tensor.matmul(
                                p[:, col : col + 128], qs, ks,
                                start=False, stop=True, skip_group_check=True,
                            )
                        # ---- softmax (masked entries exp -> 0) ----
                        e2 = mpool.tile([128, 1024], BF16, tag="e2")
                        for hh in range(2):
                            nc.scalar.activation(
                                e2[:, hh * 512 : (hh + 1) * 512], psc[hh][:],
                                EXP, bias=0.0, scale=float(1.0 / SCALE),
                            )
                        den = spool.tile([128, 8], F32, tag="den")
                        for hh in range(2):
                            nc.vector.tensor_reduce(
                                den[:, hh * 4 : (hh + 1) * 4],
                                e2[:, hh * 512 : (hh + 1) * 512].rearrange(
                                    "p (j n) -> p j n", j=4
                                ),
                                axis=mybir.AxisListType.X,
                                op=mybir.AluOpType.add,
                            )
                        r8 = spool.tile([128, 8], F32, tag="r8")
                        nc.vector.reciprocal(r8[:], den[:])
                        e2n = mpool.tile([128, 1024], BF16, tag="e2n")
                        for j in range(8):
                            nc.vector.tensor_scalar_mul(
                                e2n[:, j * 128 : (j + 1) * 128],
                                e2[:, j * 128 : (j + 1) * 128],
                                r8[:, j : j + 1],
                            )
                        # ---- attn^T and V_pack via PE transposes ----
                        pe_t = ps_e.tile([128, 1024], BF16, tag="pe_t")
                        pv_t = ps_v.tile([128, 1024], BF16, tag="pv_t")
                        for j in range(8):
                            grp = q * 8 + j
                            nc.tensor.transpose(
                                pe_t[:, j * 128 : (j + 1) * 128],
                                e2n[:, j * 128 : (j + 1) * 128],
                                ident[:],
                            )
                            nc.tensor.transpose(
                                pv_t[:, j * 128 : (j + 1) * 128],
                                vt[:, grp].rearrange("d g b -> d (g b)"),
                                ident[:],
                            )
                        eT = mpool.tile([128, 1024], BF16, tag="eT")
                        vp = mpool.tile([128, 1024], BF16, tag="vp")
                        nc.vector.tensor_copy(eT[:], pe_t[:])
                        nc.vector.tensor_copy(vp[:], pv_t[:])
                        # ---- context^T per group ----
                        pc = [
                            ps_ct.tile([128, 512], F32, tag="ct", name=f"pc{i}")
                            for i in range(2)
                        ]
                        for j in range(8):
                            nc.tensor.matmul(
                                pc[j // 4][:, (j % 4) * 128 : (j % 4 + 1) * 128],
                                vp[:, j * 128 : (j + 1) * 128],
                                eT[:, j * 128 : (j + 1) * 128],
                                start=True, stop=True,
                            )
                        # ct_sb[d, h*128 + b]  (b = sample within the 128-slab)
                        ct_sb = ctpool.tile([128, 1024], BF16, tag="ct_sb")
                        ct_v = ct_sb[:].rearrange(
                            "d (h j2 j b) -> d j2 j b h", h=8, j2=2, j=4
                        )
                        for hh in range(2):
                            nc.vector.tensor_copy(
                                ct_v[:, hh],
                                pc[hh][:].rearrange("p (j b h) -> p j b h", j=4, b=16),
                            )
                        # ---- output projection ----
                        for half in range(2):
                            po = ps_work.tile([128, 512], F32, tag="work")
                            for hc in range(8):
                                nc.tensor.matmul(
                                    po[:],
                                    ct_sb[:, hc * 128 : (hc + 1) * 128],
                                    wo[:, hc, half * 512 : (half + 1) * 512],
                                    start=(hc == 0),
                                    stop=(hc == 7),
                                )
                            o_sb = opool.tile([128, 512], F32, tag="osb")
                            nc.scalar.copy(o_sb[:], po[:])
                            nc.sync.dma_start(
                                out=out[
                                    r0 + q * 128 : r0 + q * 128 + 128,
                                    half * 512 : (half + 1) * 512,
                                ],
                                in_=o_sb[:],
                            )

            if TIME_LOOP_N:
                with tc.For_i(0, TIME_LOOP_N, 1) as _iv:
                    emit_all()
            else:
                emit_all()
    nc.compile()
    return nc


def make_core_inputs(inputs):
    """Full-size numpy inputs -> per-core in_maps (bf16, batch-sharded)."""
    from ml_dtypes import bfloat16

    f1 = np.ascontiguousarray(np.asarray(inputs["feat1"], np.float32)).astype(bfloat16)
    f2 = np.ascontiguousarray(np.asarray(inputs["feat2"], np.float32)).astype(bfloat16)
    ws = {
        k: np.ascontiguousarray(np.asarray(inputs[k], np.float32)).astype(bfloat16)
        for k in ("Wq", "Wk", "Wv", "Wo")
    }
    in_maps = []
    for c in range(NCORES):
        sl = slice(c * B_LOC, (c + 1) * B_LOC)
        m = {"feat1": np.ascontiguousarray(f1[sl]), "feat2": np.ascontiguousarray(f2[sl])}
        m.update(ws)
        in_maps.append(m)
    return in_maps


def _numpy_fallback(feat1, feat2, Wq, bq, Wk, bk, Wv, bv, Wo, bo):
    def sm(x):
        x = x - x.max(-1, keepdims=True)
        e = np.exp(x)
        return e / e.sum(-1, keepdims=True)

    b = feat1.shape[0]
    Q = (feat1 @ Wq + bq).reshape(b, H, HD)
    K = (feat2 @ Wk + bk).reshape(b, H, HD)
    V = (feat2 @ Wv + bv).reshape(b, H, HD)
    s = np.einsum("bhd,bgd->bhg", Q, K) / SCALE
    a = sm(s)
    ctx = np.einsum("bhg,bgd->bhd", a, V).reshape(b, DIM)
    return (ctx @ Wo + bo).astype(np.float32)


def kernel(feat1, feat2, Wq, bq, Wk, bk, Wv, bv, Wo, bo):
    feat1 = np.asarray(feat1, dtype=np.float32)
    feat2 = np.asarray(feat2, dtype=np.float32)
    Wq, Wk, Wv, Wo = (np.asarray(x, dtype=np.float32) for x in (Wq, Wk, Wv, Wo))
    bq, bk, bv, bo = (np.asarray(x, dtype=np.float32) for x in (bq, bk, bv, bo))
    if any(np.abs(x).max() > 0 for x in (bq, bk, bv, bo) if x.size):
        return _numpy_fallback(feat1, feat2, Wq, bq, Wk, bk, Wv, bv, Wo, bo)

    from concourse.bass_utils import run_bass_kernel_spmd

    if "nc" not in _nc_cache:
        _nc_cache["nc"] = build_nc()
    nc = _nc_cache["nc"]

    in_maps = make_core_inputs(
        {"feat1": feat1, "feat2": feat2, "Wq": Wq, "Wk": Wk, "Wv": Wv, "Wo": Wo}
    )
    res = run_bass_kernel_spmd(nc, in_maps, list(range(NCORES)))
    return np.concatenate([res.results[c]["out"] for c in range(NCORES)], axis=0)


# revision 13
# speedup vs baseline: 4.1297x; 1.9141x over previous
"""CrossAttentionFusion TRN2 kernel: 8-core data-parallel, fully fused Bass/Tile.

Per core (B_loc = 2048), one pass over the data, slab512 pipeline:
  x1t/x2t  <- DMA-XBAR transposes of bf16 feats (HBM -> SBUF, no PE time)
  qt/kt/vt <- form-A projections [d, head, b] (bf16 stationary W chunks)
  middle, per 128 samples (8 groups of 16 samples x 8 heads = 128 partitions):
    psc[(b16,h), (g,b16')] = mask-preload (rank-17 matmul, -32768 off-sample)
                             + Q.K scores (bf16, one 128x128 matmul per group)
    e2  = exp(psc/sqrt(128))  (ACT, psum->sbuf bf16; masked entries -> 0)
    den = per-group row sums (DVE tensor_reduce), r = 1/den (DVE)
    e2n = e2 * r  (DVE tensor_scalar per group)
    eT, vp = PE transposes of e2n and vt slices (bf16)
    pct[d, (b16,h)] = vp^T @ eT  (one matmul per group)
    ct_sb[d, h*128+b] <- pct (reordered DVE copy, bf16)
  out = ct @ Wo (form B: stationary ct chunks, moving bf16 Wo), ACT copy, DMA out.

All matmuls run bf16 (1 cycle/row); host converts inputs to bf16 (verified
max rel err 5.1e-3 vs the fp32 reference on the seed-0 inputs).
"""

import sys

sys.path.insert(0, "/opt/trn_rl_repo")

import numpy as np
import concourse.bacc as bacc
import concourse.mybir as mybir
import concourse.tile as tile
from concourse.masks import make_identity

B = 16384
DIM = 1024
H = 8
HD = 128
SCALE = float(np.sqrt(HD))
NCORES = 8
B_LOC = B // NCORES  # 2048
SLAB = 512
NSLAB = B_LOC // SLAB  # 4

F32 = mybir.dt.float32
BF16 = mybir.dt.bfloat16
BIG = 32768.0  # additive mask magnitude (exact in bf16; exp(-BIG/11.3) == 0)

_nc_cache = {}
TIME_LOOP_N = None  # when set, wraps the whole compute in a HW For_i loop (timing)

import os
PROBE = os.environ.get("KERNEL_PROBE", "")  # timing ablations; "" for real kernel


def build_nc():
    nc = bacc.Bacc(None)
    feat1 = nc.declare_dram_parameter("feat1", [B_LOC, DIM], BF16, isOutput=False)
    feat2 = nc.declare_dram_parameter("feat2", [B_LOC, DIM], BF16, isOutput=False)
    Wq = nc.declare_dram_parameter("Wq", [DIM, DIM], BF16, isOutput=False)
    Wk = nc.declare_dram_parameter("Wk", [DIM, DIM], BF16, isOutput=False)
    Wv = nc.declare_dram_parameter("Wv", [DIM, DIM], BF16, isOutput=False)
    Wo = nc.declare_dram_parameter("Wo", [DIM, DIM], BF16, isOutput=False)
    out = nc.declare_dram_parameter("out", [B_LOC, DIM], F32, isOutput=True)

    w_view = lambda W: W.rearrange("(c p) n -> p c n", p=128)  # (f-part, fchunk, o)
    EXP = mybir.ActivationFunctionType.Exp

    with tile.TileContext(nc) as tc:
        with (
            tc.tile_pool(name="const", bufs=1) as cpool,
            tc.tile_pool(name="w", bufs=1) as wpool,
            tc.tile_pool(name="xt", bufs=2) as xpool,
            tc.tile_pool(name="qkv", bufs=2) as qkvpool,
            tc.tile_pool(name="mid", bufs=2) as mpool,
            tc.tile_pool(name="sm", bufs=2) as spool,
            tc.tile_pool(name="ct", bufs=2) as ctpool,
            tc.tile_pool(name="osb", bufs=2) as opool,
            tc.tile_pool(name="ps_work", bufs=2, space="PSUM") as ps_work,
            tc.tile_pool(name="ps_sc", bufs=2, space="PSUM") as ps_sc,
            tc.tile_pool(name="ps_e", bufs=1, space="PSUM") as ps_e,
            tc.tile_pool(name="ps_v", bufs=1, space="PSUM") as ps_v,
            tc.tile_pool(name="ps_ct", bufs=2, space="PSUM") as ps_ct,
        ):
            ident = cpool.tile([128, 128], BF16, tag="ident")
            make_identity(nc, ident)
            # rank-17 factors of the additive mask:
            #   Madd[(b,h), (g,b')] = -BIG * (b != b') = -BIG*1 + BIG*[b == b']
            mask_l = cpool.tile([32, 128], BF16, tag="mask_l")
            mask_r = cpool.tile([32, 512], BF16, tag="mask_r")
            nc.gpsimd.memset(mask_l[:], 0.0)
            nc.gpsimd.memset(mask_r[:], 0.0)
            # rows 1+c: BIG where m//8 == c  (iota = k - 1 - m//8 == 0)
            nc.gpsimd.affine_select(
                out=mask_l[:].rearrange("p (c w) -> p c w", w=8),
                in_=mask_l[:].rearrange("p (c w) -> p c w", w=8),
                compare_op=mybir.AluOpType.not_equal,
                fill=BIG,
                base=-1,
                pattern=[[-1, 16], [0, 8]],
                channel_multiplier=1,
            )
            # rows 1+c: 1.0 where n%16 == c  (iota = k - 1 - n%16 == 0)
            nc.gpsimd.affine_select(
                out=mask_r[:].rearrange("p (k s) -> p k s", s=16),
                in_=mask_r[:].rearrange("p (k s) -> p k s", s=16),
                compare_op=mybir.AluOpType.not_equal,
                fill=1.0,
                base=-1,
                pattern=[[0, 32], [-1, 16]],
                channel_multiplier=1,
            )
            nc.gpsimd.memset(mask_l[0:1, :], -BIG)
            nc.gpsimd.memset(mask_r[0:1, :], 1.0)
            ct_const = cpool.tile([128, 1024], BF16, tag="ct_const")
            nc.gpsimd.memset(ct_const[:], 0.001)

            wq = wpool.tile([128, 8, DIM], BF16, tag="wq")
            wk = wpool.tile([128, 8, DIM], BF16, tag="wk")
            wv = wpool.tile([128, 8, DIM], BF16, tag="wv")
            wo = wpool.tile([128, 8, DIM], BF16, tag="wo")
            nc.gpsimd.dma_start(out=wq[:], in_=w_view(Wq))
            nc.gpsimd.dma_start(out=wk[:], in_=w_view(Wk))
            nc.gpsimd.dma_start(out=wv[:], in_=w_view(Wv))
            nc.gpsimd.dma_start(out=wo[:], in_=w_view(Wo))

            def emit_all():
                for s in range(NSLAB):
                    r0 = s * SLAB
                    # ---- input transposes via DMA XBAR (bf16) ----
                    x1t = xpool.tile([128, 8, SLAB], BF16, tag="x1t")
                    x2t = xpool.tile([128, 8, SLAB], BF16, tag="x2t")
                    for c in range(8):
                        if PROBE == "noxbar":  # straight DMA, same bytes (wrong data)
                            nc.sync.dma_start(
                                out=x1t[:, c, :], in_=feat1[r0 : r0 + 128, c // 2 * 512 : c // 2 * 512 + 512]
                            )
                            nc.sync.dma_start(
                                out=x2t[:, c, :], in_=feat2[r0 : r0 + 128, c // 2 * 512 : c // 2 * 512 + 512]
                            )
                            continue
                        nc.sync.dma_start_transpose(
                            x1t[:, c, :], feat1[r0 : r0 + SLAB, c * 128 : (c + 1) * 128]
                        )
                        nc.sync.dma_start_transpose(
                            x2t[:, c, :], feat2[r0 : r0 + SLAB, c * 128 : (c + 1) * 128]
                        )
                    # ---- QKV projections, form A ----
                    # group-local layouts so every 16-sample group slice is a
                    # contiguous 128 columns (matmul moving APs need 1 free dim):
                    #   qt: [d, grp, b16, h]   kt/vt: [d, grp, g, b16]
                    qt = qkvpool.tile([128, 32, 16, 8], BF16, tag="qt")
                    kt = qkvpool.tile([128, 32, 8, 16], BF16, tag="kt")
                    vt = qkvpool.tile([128, 32, 8, 16], BF16, tag="vt")
                    for (w, xt, dst, qlike) in (
                        (wq, x1t, qt, True),
                        (wk, x2t, kt, False),
                        (wv, x2t, vt, False),
                    ):
                        for oc in range(8):
                            ps = ps_work.tile([128, SLAB], F32, tag="work")
                            for fc in range(8):
                                nc.tensor.matmul(
                                    ps[:],
                                    w[:, fc, oc * 128 : (oc + 1) * 128],
                                    xt[:, fc, :],
                                    start=(fc == 0),
                                    stop=(fc == 7),
                                )
                            # psum cols are b = grp*16 + b16
                            if qlike:
                                dst_ap = dst[:, :, :, oc]
                            else:
                                dst_ap = dst[:, :, oc, :]
                            nc.scalar.copy(
                                dst_ap, ps[:].rearrange("p (g b) -> p g b", b=16)
                            )

                    for q in range(4):
                        if PROBE == "nomiddle":
                            ct_sb = ct_const
                            for half in range(2):
                                po = ps_work.tile([128, 512], F32, tag="work")
                                for hc in range(8):
                                    nc.tensor.matmul(
                                        po[:],
                                        ct_sb[:, hc * 128 : (hc + 1) * 128],
                                        wo[:, hc, half * 512 : (half + 1) * 512],
                                        start=(hc == 0),
                                        stop=(hc == 7),
                                    )
                                o_sb = opool.tile([128, 512], F32, tag="osb")
                                nc.scalar.copy(o_sb[:], po[:])
                                nc.sync.dma_start(
                                    out=out[
                                        r0 + q * 128 : r0 + q * 128 + 128,
                                        half * 512 : (half + 1) * 512,
                                    ],
                                    in_=o_sb[:],
                                )
                            continue
                        # ---- scores + mask ----
                        psc = [
                            ps_sc.tile([128, 512], F32, tag="sc", name=f"psc{i}")
                            for i in range(2)
                        ]
                        for p in psc:
                            nc.tensor.matmul(
                                p[:], mask_l[:], mask_r[:],
                                start=True, stop=False, skip_group_check=True,
                            )
                        for j in range(8):
                            grp = q * 8 + j
                            qs = qt[:, grp].rearrange("d b h -> d (b h)")
                            ks = kt[:, grp].rearrange("d g b -> d (g b)")
                            p = psc[j // 4]
                            col = (j % 4) * 128
                            nc.tensor.matmul(
                                p[:, col : col + 128], qs, ks,
                                start=False, stop=True, skip_group_check=True,
                            )
                        # ---- softmax (masked entries exp -> 0) ----
                        e2 = mpool.tile([128, 1024], BF16, tag="e2")
                        for hh in range(2):
                            nc.scalar.activation(
                                e2[:, hh * 512 : (hh + 1) * 512], psc[hh][:],
                                EXP, bias=0.0, scale=float(1.0 / SCALE),
                            )
                        den = spool.tile([128, 8], F32, tag="den")
                        for hh in range(2):
                            nc.vector.tensor_reduce(
                                den[:, hh * 4 : (hh + 1) * 4],
                                e2[:, hh * 512 : (hh + 1) * 512].rearrange(
                                    "p (j n) -> p j n", j=4
                                ),
                                axis=mybir.AxisListType.X,
                                op=mybir.AluOpType.add,
                            )
                        r8 = spool.tile([128, 8], F32, tag="r8")
                        nc.vector.reciprocal(r8[:], den[:])
                        e2n = mpool.tile([128, 1024], BF16, tag="e2n")
                        for j in range(8):
                            nc.vector.tensor_scalar_mul(
                                e2n[:, j * 128 : (j + 1) * 128],
                                e2[:, j * 128 : (j + 1) * 128],
                                r8[:, j : j + 1],
                            )
                        # ---- attn^T and V_pack via PE transposes ----
                        pe_t = ps_e.tile([128, 1024], BF16, tag="pe_t")
                        pv_t = ps_v.tile([128, 1024], BF16, tag="pv_t")
                        for j in range(8):
                            grp = q * 8 + j
                            nc.tensor.transpose(
                                pe_t[:, j * 128 : (j + 1) * 128],
                                e2n[:, j * 128 : (j + 1) * 128],
                                ident[:],
                            )
                            nc.tensor.transpose(
                                pv_t[:, j * 128 : (j + 1) * 128],
                                vt[:, grp].rearrange("d g b -> d (g b)"),
                                ident[:],
                            )
                        eT = mpool.tile([128, 1024], BF16, tag="eT")
                        vp = mpool.tile([128, 1024], BF16, tag="vp")
                        nc.vector.tensor_copy(eT[:], pe_t[:])
                        nc.vector.tensor_copy(vp[:], pv_t[:])
                        # ---- context^T per group ----
                        pc = [
                            ps_ct.tile([128, 512], F32, tag="ct", name=f"pc{i}")
                            for i in range(2)
                        ]
                        for j in range(8):
                            nc.tensor.matmul(
                                pc[j // 4][:, (j % 4) * 128 : (j % 4 + 1) * 128],
                                vp[:, j * 128 : (j + 1) * 128],
                                eT[:, j * 128 : (j + 1) * 128],
                                start=True, stop=True,
                            )
                        # ct_sb[d, h*128 + b]  (b = sample within the 128-slab)
                        ct_sb = ctpool.tile([128, 1024], BF16, tag="ct_sb")
                        ct_v = ct_sb[:].rearrange(
                            "d (h j2 j b) -> d j2 j b h", h=8, j2=2, j=4
                        )
                        for hh in range(2):
                            nc.vector.tensor_copy(
                                ct_v[:, hh],
                                pc[hh][:].rearrange("p (j b h) -> p j b h", j=4, b=16),
                            )
                        # ---- output projection ----
                        for half in range(2):
                            po = ps_work.tile([128, 512], F32, tag="work")
                            for hc in range(8):
                                nc.tensor.matmul(
                                    po[:],
                                    ct_sb[:, hc * 128 : (hc + 1) * 128],
                                    wo[:, hc, half * 512 : (half + 1) * 512],
                                    start=(hc == 0),
                                    stop=(hc == 7),
                                )
                            o_sb = opool.tile([128, 512], F32, tag="osb")
                            nc.scalar.copy(o_sb[:], po[:])
                            nc.sync.dma_start(
                                out=out[
                                    r0 + q * 128 : r0 + q * 128 + 128,
                                    half * 512 : (half + 1) * 512,
                                ],
                                in_=o_sb[:],
                            )

            if TIME_LOOP_N:
                with tc.For_i(0, TIME_LOOP_N, 1) as _iv:
                    emit_all()
            else:
                emit_all()
    nc.compile()
    return nc


def make_core_inputs(inputs):
    """Full-size numpy inputs -> per-core in_maps (bf16, batch-sharded)."""
    from ml_dtypes import bfloat16

    f1 = np.ascontiguousarray(np.asarray(inputs["feat1"], np.float32)).astype(bfloat16)
    f2 = np.ascontiguousarray(np.asarray(inputs["feat2"], np.float32)).astype(bfloat16)
    ws = {
        k: np.ascontiguousarray(np.asarray(inputs[k], np.float32)).astype(bfloat16)
        for k in ("Wq", "Wk", "Wv", "Wo")
    }
    in_maps = []
    for c in range(NCORES):
        sl = slice(c * B_LOC, (c + 1) * B_LOC)
        m = {"feat1": np.ascontiguousarray(f1[sl]), "feat2": np.ascontiguousarray(f2[sl])}
        m.update(ws)
        in_maps.append(m)
    return in_maps


def _numpy_fallback(feat1, feat2, Wq, bq, Wk, bk, Wv, bv, Wo, bo):
    def sm(x):
        x = x - x.max(-1, keepdims=True)
        e = np.exp(x)
        return e / e.sum(-1, keepdims=True)

    b = feat1.shape[0]
    Q = (feat1 @ Wq + bq).reshape(b, H, HD)
    K = (feat2 @ Wk + bk).reshape(b, H, HD)
    V = (feat2 @ Wv + bv).reshape(b, H, HD)
    s = np.einsum("bhd,bgd->bhg", Q, K) / SCALE
    a = sm(s)
    ctx = np.einsum("bhg,bgd->bhd", a, V).reshape(b, DIM)
    return (ctx @ Wo + bo).astype(np.float32)


def kernel(feat1, feat2, Wq, bq, Wk, bk, Wv, bv, Wo, bo):
    feat1 = np.asarray(feat1, dtype=np.float32)
    feat2 = np.asarray(feat2, dtype=np.float32)
    Wq, Wk, Wv, Wo = (np.asarray(x, dtype=np.float32) for x in (Wq, Wk, Wv, Wo))
    bq, bk, bv, bo = (np.asarray(x, dtype=np.float32) for x in (bq, bk, bv, bo))
    if any(np.abs(x).max() > 0 for x in (bq, bk, bv, bo) if x.size):
        return _numpy_fallback(feat1, feat2, Wq, bq, Wk, bk, Wv, bv, Wo, bo)

    from concourse.bass_utils import run_bass_kernel_spmd

    if "nc" not in _nc_cache:
        _nc_cache["nc"] = build_nc()
    nc = _nc_cache["nc"]

    in_maps = make_core_inputs(
        {"feat1": feat1, "feat2": feat2, "Wq": Wq, "Wk": Wk, "Wv": Wv, "Wo": Wo}
    )
    res = run_bass_kernel_spmd(nc, in_maps, list(range(NCORES)))
    return np.concatenate([res.results[c]["out"] for c in range(NCORES)], axis=0)


# revision 15
# speedup vs baseline: 5.2801x; 1.2786x over previous
"""CrossAttentionFusion TRN2 kernel: 8-core data-parallel, fully fused Bass/Tile.

Per core (B_loc = 2048), one pass over the data, slab512 pipeline:
  x1t/x2t  <- DMA-XBAR transposes of bf16 feats (HBM -> SBUF, no PE time)
  qt/kt/vt <- form-A projections [d, head, b] (bf16 stationary W chunks)
  middle, per 128 samples (8 groups of 16 samples x 8 heads = 128 partitions):
    psc[(b16,h), (g,b16')] = mask-preload (rank-17 matmul, -32768 off-sample)
                             + Q.K scores (bf16, one 128x128 matmul per group)
    e2  = exp(psc/sqrt(128))  (ACT, psum->sbuf bf16; masked entries -> 0)
    den = per-group row sums (DVE tensor_reduce), r = 1/den (DVE)
    e2n = e2 * r  (DVE tensor_scalar per group)
    eT, vp = PE transposes of e2n and vt slices (bf16)
    pct[d, (b16,h)] = vp^T @ eT  (one matmul per group)
    ct_sb[d, h*128+b] <- pct (reordered DVE copy, bf16)
  out = ct @ Wo (form B: stationary ct chunks, moving bf16 Wo), ACT copy, DMA out.

All matmuls run bf16 (1 cycle/row); host converts inputs to bf16 (verified
max rel err 5.1e-3 vs the fp32 reference on the seed-0 inputs).
"""

import sys

sys.path.insert(0, "/opt/trn_rl_repo")

import numpy as np
import concourse.bacc as bacc
import concourse.mybir as mybir
import concourse.tile as tile
from concourse.masks import make_identity

B = 16384
DIM = 1024
H = 8
HD = 128
SCALE = float(np.sqrt(HD))
NCORES = 8
B_LOC = B // NCORES  # 2048
SLAB = 512
NSLAB = B_LOC // SLAB  # 4

F32 = mybir.dt.float32
BF16 = mybir.dt.bfloat16
BIG = 32768.0  # additive mask magnitude (exact in bf16; exp(-BIG/11.3) == 0)

_nc_cache = {}
TIME_LOOP_N = None  # when set, wraps the whole compute in a HW For_i loop (timing)

import os
PROBE = os.environ.get("KERNEL_PROBE", "")  # timing ablations; "" for real kernel


def build_nc():
    nc = bacc.Bacc(None)
    feat1 = nc.declare_dram_parameter("feat1", [B_LOC, DIM], BF16, isOutput=False)
    feat2 = nc.declare_dram_parameter("feat2", [B_LOC, DIM], BF16, isOutput=False)
    Wq = nc.declare_dram_parameter("Wq", [DIM, DIM], BF16, isOutput=False)
    Wk = nc.declare_dram_parameter("Wk", [DIM, DIM], BF16, isOutput=False)
    Wv = nc.declare_dram_parameter("Wv", [DIM, DIM], BF16, isOutput=False)
    Wo = nc.declare_dram_parameter("Wo", [DIM, DIM], BF16, isOutput=False)
    out = nc.declare_dram_parameter("out", [B_LOC, DIM], F32, isOutput=True)

    w_view = lambda W: W.rearrange("(c p) n -> p c n", p=128)  # (f-part, fchunk, o)
    EXP = mybir.ActivationFunctionType.Exp

    with tile.TileContext(nc) as tc:
        with (
            tc.tile_pool(name="const", bufs=1) as cpool,
            tc.tile_pool(name="w", bufs=1) as wpool,
            tc.tile_pool(name="xt", bufs=2) as xpool,
            tc.tile_pool(name="qkv", bufs=2) as qkvpool,
            tc.tile_pool(name="mid", bufs=2) as mpool,
            tc.tile_pool(name="sm", bufs=2) as spool,
            tc.tile_pool(name="ct", bufs=2) as ctpool,
            tc.tile_pool(name="osb", bufs=2) as opool,
            tc.tile_pool(name="ps_work", bufs=2, space="PSUM") as ps_work,
            tc.tile_pool(name="ps_sc", bufs=2, space="PSUM") as ps_sc,
            tc.tile_pool(name="ps_e", bufs=1, space="PSUM") as ps_e,
            tc.tile_pool(name="ps_v", bufs=1, space="PSUM") as ps_v,
            tc.tile_pool(name="ps_ct", bufs=2, space="PSUM") as ps_ct,
        ):
            ident = cpool.tile([128, 128], BF16, tag="ident")
            make_identity(nc, ident)
            # rank-17 factors of the additive mask:
            #   Madd[(b,h), (g,b')] = -BIG * (b != b') = -BIG*1 + BIG*[b == b']
            mask_l = cpool.tile([32, 128], BF16, tag="mask_l")
            mask_r = cpool.tile([32, 512], BF16, tag="mask_r")
            nc.gpsimd.memset(mask_l[:], 0.0)
            nc.gpsimd.memset(mask_r[:], 0.0)
            # rows 1+c: BIG where m//8 == c  (iota = k - 1 - m//8 == 0)
            nc.gpsimd.affine_select(
                out=mask_l[:].rearrange("p (c w) -> p c w", w=8),
                in_=mask_l[:].rearrange("p (c w) -> p c w", w=8),
                compare_op=mybir.AluOpType.not_equal,
                fill=BIG,
                base=-1,
                pattern=[[-1, 16], [0, 8]],
                channel_multiplier=1,
            )
            # rows 1+c: 1.0 where n%16 == c  (iota = k - 1 - n%16 == 0)
            nc.gpsimd.affine_select(
                out=mask_r[:].rearrange("p (k s) -> p k s", s=16),
                in_=mask_r[:].rearrange("p (k s) -> p k s", s=16),
                compare_op=mybir.AluOpType.not_equal,
                fill=1.0,
                base=-1,
                pattern=[[0, 32], [-1, 16]],
                channel_multiplier=1,
            )
            nc.gpsimd.memset(mask_l[0:1, :], -BIG)
            nc.gpsimd.memset(mask_r[0:1, :], 1.0)
            ct_const = cpool.tile([128, 1024], BF16, tag="ct_const")
            nc.gpsimd.memset(ct_const[:], 0.001)

            wq = wpool.tile([128, 8, DIM], BF16, tag="wq")
            wk = wpool.tile([128, 8, DIM], BF16, tag="wk")
            wv = wpool.tile([128, 8, DIM], BF16, tag="wv")
            wo = wpool.tile([128, 8, DIM], BF16, tag="wo")
            nc.gpsimd.dma_start(out=wq[:], in_=w_view(Wq))
            nc.gpsimd.dma_start(out=wk[:], in_=w_view(Wk))
            nc.gpsimd.dma_start(out=wv[:], in_=w_view(Wv))
            nc.gpsimd.dma_start(out=wo[:], in_=w_view(Wo))

            def emit_all():
                for s in range(NSLAB):
                    r0 = s * SLAB
                    # ---- input transposes via DMA XBAR (bf16) ----
                    x1t = xpool.tile([128, 8, SLAB], BF16, tag="x1t")
                    x2t = xpool.tile([128, 8, SLAB], BF16, tag="x2t")
                    for c in range(8):
                        if PROBE == "noxbar":  # straight DMA, same bytes (wrong data)
                            nc.sync.dma_start(
                                out=x1t[:, c, :], in_=feat1[r0 : r0 + 128, c % 2 * 512 : c % 2 * 512 + 512]
                            )
                            nc.sync.dma_start(
                                out=x2t[:, c, :], in_=feat2[r0 : r0 + 128, c % 2 * 512 : c % 2 * 512 + 512]
                            )
                            continue
                        nc.sync.dma_start_transpose(
                            x1t[:, c, :], feat1[r0 : r0 + SLAB, c * 128 : (c + 1) * 128]
                        )
                        nc.sync.dma_start_transpose(
                            x2t[:, c, :], feat2[r0 : r0 + SLAB, c * 128 : (c + 1) * 128]
                        )
                    # ---- QKV projections, form A ----
                    # group-local layouts so every 16-sample group slice is a
                    # contiguous 128 columns (matmul moving APs need 1 free dim):
                    #   qt: [d, grp, b16, h]   kt/vt: [d, grp, g, b16]
                    qt = qkvpool.tile([128, 32, 16, 8], BF16, tag="qt")
                    kt = qkvpool.tile([128, 32, 8, 16], BF16, tag="kt")
                    vt = qkvpool.tile([128, 32, 8, 16], BF16, tag="vt")
                    for (w, xt, dst, qlike) in (
                        (wq, x1t, qt, True),
                        (wk, x2t, kt, False),
                        (wv, x2t, vt, False),
                    ):
                        for oc in range(8):
                            ps = ps_work.tile([128, SLAB], F32, tag="work")
                            for fc in range(8):
                                nc.tensor.matmul(
                                    ps[:],
                                    w[:, fc, oc * 128 : (oc + 1) * 128],
                                    xt[:, fc, :],
                                    start=(fc == 0),
                                    stop=(fc == 7),
                                )
                            if PROBE == "mm_only":
                                continue
                            # psum cols are b = grp*16 + b16
                            if qlike:
                                dst_ap = dst[:, :, :, oc]
                            else:
                                dst_ap = dst[:, :, oc, :]
                            nc.scalar.copy(
                                dst_ap, ps[:].rearrange("p (g b) -> p g b", b=16)
                            )
                    if PROBE == "mm_only":
                        continue

                    for q in range(4):
                        if PROBE == "nomiddle":
                            ct_sb = ct_const
                            for half in range(2):
                                po = ps_work.tile([128, 512], F32, tag="work")
                                for hc in range(8):
                                    nc.tensor.matmul(
                                        po[:],
                                        ct_sb[:, hc * 128 : (hc + 1) * 128],
                                        wo[:, hc, half * 512 : (half + 1) * 512],
                                        start=(hc == 0),
                                        stop=(hc == 7),
                                    )
                                o_sb = opool.tile([128, 512], F32, tag="osb")
                                nc.scalar.copy(o_sb[:], po[:])
                                nc.sync.dma_start(
                                    out=out[
                                        r0 + q * 128 : r0 + q * 128 + 128,
                                        half * 512 : (half + 1) * 512,
                                    ],
                                    in_=o_sb[:],
                                )
                            continue
                        # ---- scores + mask ----
                        psc = [
                            ps_sc.tile([128, 512], F32, tag="sc", name=f"psc{i}")
                            for i in range(2)
                        ]
                        for p in psc:
                            nc.tensor.matmul(
                                p[:], mask_l[:], mask_r[:],
                                start=True, stop=False, skip_group_check=True,
                            )
                        for j in range(8):
                            grp = q * 8 + j
                            qs = qt[:, grp].rearrange("d b h -> d (b h)")
                            ks = kt[:, grp].rearrange("d g b -> d (g b)")
                            p = psc[j // 4]
                            col = (j % 4) * 128
                            nc.tensor.matmul(
                                p[:, col : col + 128], qs, ks,
                                start=False, stop=True, skip_group_check=True,
                            )
                        # ---- softmax (masked entries exp -> 0) ----
                        e2 = mpool.tile([128, 1024], BF16, tag="e2")
                        for hh in range(2):
                            nc.scalar.activation(
                                e2[:, hh * 512 : (hh + 1) * 512], psc[hh][:],
                                EXP, bias=0.0, scale=float(1.0 / SCALE),
                            )
                        den = spool.tile([128, 8], F32, tag="den")
                        for hh in range(2):
                            nc.vector.tensor_reduce(
                                den[:, hh * 4 : (hh + 1) * 4],
                                e2[:, hh * 512 : (hh + 1) * 512].rearrange(
                                    "p (j n) -> p j n", j=4
                                ),
                                axis=mybir.AxisListType.X,
                                op=mybir.AluOpType.add,
                            )
                        r8 = spool.tile([128, 8], F32, tag="r8")
                        nc.vector.reciprocal(r8[:], den[:])
                        e2n = mpool.tile([128, 1024], BF16, tag="e2n")
                        for j in range(8):
                            nc.vector.tensor_scalar_mul(
                                e2n[:, j * 128 : (j + 1) * 128],
                                e2[:, j * 128 : (j + 1) * 128],
                                r8[:, j : j + 1],
                            )
                        # ---- attn^T and V_pack via PE transposes ----
                        pe_t = ps_e.tile([128, 1024], BF16, tag="pe_t")
                        pv_t = ps_v.tile([128, 1024], BF16, tag="pv_t")
                        for j in range(8):
                            grp = q * 8 + j
                            nc.tensor.transpose(
                                pe_t[:, j * 128 : (j + 1) * 128],
                                e2n[:, j * 128 : (j + 1) * 128],
                                ident[:],
                            )
                            nc.tensor.transpose(
                                pv_t[:, j * 128 : (j + 1) * 128],
                                vt[:, grp].rearrange("d g b -> d (g b)"),
                                ident[:],
                            )
                        eT = mpool.tile([128, 1024], BF16, tag="eT")
                        vp = mpool.tile([128, 1024], BF16, tag="vp")
                        nc.vector.tensor_copy(eT[:], pe_t[:])
                        nc.vector.tensor_copy(vp[:], pv_t[:])
                        # ---- context^T per group ----
                        pc = [
                            ps_ct.tile([128, 512], F32, tag="ct", name=f"pc{i}")
                            for i in range(2)
                        ]
                        for j in range(8):
                            nc.tensor.matmul(
                                pc[j // 4][:, (j % 4) * 128 : (j % 4 + 1) * 128],
                                vp[:, j * 128 : (j + 1) * 128],
                                eT[:, j * 128 : (j + 1) * 128],
                                start=True, stop=True,
                            )
                        # ct_sb[d, h*128 + b]  (b = sample within the 128-slab)
                        ct_sb = ctpool.tile([128, 1024], BF16, tag="ct_sb")
                        ct_v = ct_sb[:].rearrange(
                            "d (h j2 j b) -> d j2 j b h", h=8, j2=2, j=4
                        )
                        for hh in range(2):
                            nc.vector.tensor_copy(
                                ct_v[:, hh],
                                pc[hh][:].rearrange("p (j b h) -> p j b h", j=4, b=16),
                            )
                        # ---- output projection ----
                        for half in range(2):
                            po = ps_work.tile([128, 512], F32, tag="work")
                            for hc in range(8):
                                nc.tensor.matmul(
                                    po[:],
                                    ct_sb[:, hc * 128 : (hc + 1) * 128],
                                    wo[:, hc, half * 512 : (half + 1) * 512],
                                    start=(hc == 0),
                                    stop=(hc == 7),
                                )
                            o_sb = opool.tile([128, 512], F32, tag="osb")
                            nc.scalar.copy(o_sb[:], po[:])
                            nc.sync.dma_start(
                                out=out[
                                    r0 + q * 128 : r0 + q * 128 + 128,
                                    half * 512 : (half + 1) * 512,
                                ],
                                in_=o_sb[:],
                            )

            if TIME_LOOP_N:
                with tc.For_i(0, TIME_LOOP_N, 1) as _iv:
                    emit_all()
            else:
                emit_all()
    nc.compile()
    return nc


def make_core_inputs(inputs):
    """Full-size numpy inputs -> per-core in_maps (bf16, batch-sharded)."""
    from ml_dtypes import bfloat16

    f1 = np.ascontiguousarray(np.asarray(inputs["feat1"], np.float32)).astype(bfloat16)
    f2 = np.ascontiguousarray(np.asarray(inputs["feat2"], np.float32)).astype(bfloat16)
    ws = {
        k: np.ascontiguousarray(np.asarray(inputs[k], np.float32)).astype(bfloat16)
        for k in ("Wq", "Wk", "Wv", "Wo")
    }
    in_maps = []
    for c in range(NCORES):
        sl = slice(c * B_LOC, (c + 1) * B_LOC)
        m = {"feat1": np.ascontiguousarray(f1[sl]), "feat2": np.ascontiguousarray(f2[sl])}
        m.update(ws)
        in_maps.append(m)
    return in_maps


def _numpy_fallback(feat1, feat2, Wq, bq, Wk, bk, Wv, bv, Wo, bo):
    def sm(x):
        x = x - x.max(-1, keepdims=True)
        e = np.exp(x)
        return e / e.sum(-1, keepdims=True)

    b = feat1.shape[0]
    Q = (feat1 @ Wq + bq).reshape(b, H, HD)
    K = (feat2 @ Wk + bk).reshape(b, H, HD)
    V = (feat2 @ Wv + bv).reshape(b, H, HD)
    s = np.einsum("bhd,bgd->bhg", Q, K) / SCALE
    a = sm(s)
    ctx = np.einsum("bhg,bgd->bhd", a, V).reshape(b, DIM)
    return (ctx @ Wo + bo).astype(np.float32)


def kernel(feat1, feat2, Wq, bq, Wk, bk, Wv, bv, Wo, bo):
    feat1 = np.asarray(feat1, dtype=np.float32)
    feat2 = np.asarray(feat2, dtype=np.float32)
    Wq, Wk, Wv, Wo = (np.asarray(x, dtype=np.float32) for x in (Wq, Wk, Wv, Wo))
    bq, bk, bv, bo = (np.asarray(x, dtype=np.float32) for x in (bq, bk, bv, bo))
    if any(np.abs(x).max() > 0 for x in (bq, bk, bv, bo) if x.size):
        return _numpy_fallback(feat1, feat2, Wq, bq, Wk, bk, Wv, bv, Wo, bo)

    from concourse.bass_utils import run_bass_kernel_spmd

    if "nc" not in _nc_cache:
        _nc_cache["nc"] = build_nc()
    nc = _nc_cache["nc"]

    in_maps = make_core_inputs(
        {"feat1": feat1, "feat2": feat2, "Wq": Wq, "Wk": Wk, "Wv": Wv, "Wo": Wo}
    )
    res = run_bass_kernel_spmd(nc, in_maps, list(range(NCORES)))
    return np.concatenate([res.results[c]["out"] for c in range(NCORES)], axis=0)
